# revision 4
# baseline (speedup 1.0000x reference)
"""Distributed Trainium2 kernel for nn_Aggregator (segment reduce + MLP + BN).

v2 design (8 NeuronCores, SPMD), slab-major stream:
  - Host assigns each segment to one core (snake deal by segment size).
    Each core gets its segments' edges as ONE feat-major bf16 stream
    xt [128, LT]: segments are "slots" grouped into buckets of equal padded
    length K (multiple of GRAN, zero-padded), buckets tiled into units of
    <= T_S slots laid out K-major (slab j = one edge-column per slot).
  - Per chunk, ScalarE squares the stream into a SEPARATE buffer (so the
    square runs concurrently with the sum matmuls instead of after them).
  - Per unit, TensorE accumulates sum_j slab_j (from tch) and sum_j slab_j^2
    (from the squared buffer) into PSUM via identity matmuls.
  - VectorE computes per-slot min / max by K-major log2 folds (bf16 2x).
  - Raw per-slot sums are evacuated PSUM -> SBUF bf16 by ScalarE; division
    by count is deferred to the MLP block (one VectorE mult).
  - Counts / reciprocals / degree embeddings are host-precomputed layout
    tables: no on-device count machinery.  Zero padding makes empty/pad
    slots produce h == hinv, corrected exactly in the BN sums.
  - Node MLP interleaved into the stream loop as slot blocks finalize;
    BN partial sums accumulated by ScalarE accum_out; BN sums all-reduced
    across cores; normalize + ReLU fused into one ScalarE activation.
"""

import numpy as np
import ml_dtypes

import concourse.bass as bass
import concourse.bacc as bacc
import concourse.tile as tile
import concourse.mybir as mybir
from concourse import bass_utils

BF16 = ml_dtypes.bfloat16
F32 = np.float32

NCORES = 8
D = 128
GRAN = 2             # segment length padding granularity
T_S = 512            # slots per tile (psum accumulation group)
CHUNK = 8192         # stream cols per DMA chunk
KP_MAX = CHUNK // T_S  # max slabs per piece (16)
SC = 1024            # slots per MLP/BN chunk
EPS_STD = 1e-5
EPS_BN = 1e-5
USE_ALLGATHER = False

dt = mybir.dt


# ----------------------------------------------------------------------------
# Host-side planning (layout only)
# ----------------------------------------------------------------------------

class Plan:
    pass


def make_plan(index, N):
    E = index.shape[0]
    p = Plan()
    p.E, p.N = E, N

    counts = np.bincount(index, minlength=N)
    order = np.argsort(-counts, kind="stable")
    pos = np.arange(N)
    r, q = pos // NCORES, pos % NCORES
    snake = np.where(r % 2 == 0, q, NCORES - 1 - q)
    segs_c = [order[snake == c] for c in range(NCORES)]

    # choose K bins by DP: padding cost vs per-bucket tail-tile overhead
    cmax = int(counts.max())
    hist = np.bincount(counts, minlength=cmax + 1).astype(np.int64)
    PAD_NS = 4.0        # ns of critical-engine time per padded col (per core)
    BUCK_NS = lambda K: K * 120 + 2500      # tail-tile matmul+fold overhead
    vals = [c for c in range(1, cmax + 1) if hist[c] > 0]
    nv = len(vals)
    INF = float("inf")
    dp = [0.0] + [INF] * nv
    choice = [0] * (nv + 1)
    for i in range(1, nv + 1):
        for j in range(1, i + 1):
            K = (vals[i - 1] + 1) // 2 * 2  # even round-up of bin max
            pad = sum(hist[vals[t]] * (K - vals[t])
                      for t in range(j - 1, i)) / NCORES
            cost = dp[j - 1] + pad * PAD_NS + BUCK_NS(K)
            if cost < dp[i]:
                dp[i] = cost
                choice[i] = j - 1
    bins = []
    i = nv
    while i > 0:
        j = choice[i]
        bins.append(((vals[j] if j < nv else vals[-1]), vals[i - 1]))
        i = j
    bins.reverse()
    Kmap = np.zeros(cmax + 1, np.int64)
    for lo, hi in bins:
        K = (hi + 1) // 2 * 2
        Kmap[lo:hi + 1] = K
    Kof = np.maximum(GRAN, Kmap[counts])

    allK = sorted(set(int(k) for k in np.unique(Kof)))
    S_K = {}
    for K in allK:
        m = max(int(np.sum(Kof[segs_c[c]] == K)) for c in range(NCORES))
        S_K[K] = m + (m & 1)  # even

    # buckets: K, SK, slot base, col base
    p.buckets = []
    sp = 0
    lt = 0
    for K in allK:
        SK = S_K[K]
        p.buckets.append(dict(K=K, SK=SK, base=sp, coff=lt))
        sp += SK
        lt += SK * K
    p.S = sp
    p.LT = lt

    # units (pieces): stream layout + schedule.  The first few chunks are
    # small so the first folds/matmuls start as soon as possible.
    HEAD_N, HEAD_CAP = 4, 2048
    units = []
    col = 0
    tid = 0
    for bi, b in enumerate(p.buckets):
        K, SK = b["K"], b["SK"]
        for t0 in range(0, SK, T_S):
            Tt = min(T_S, SK - t0)
            k0 = 0
            while k0 < K:
                cap = HEAD_CAP if col < HEAD_N * HEAD_CAP else KP_MAX * T_S
                Kp = min(max(1, cap // Tt), KP_MAX, K - k0)
                units.append(dict(col=col, Kp=Kp, Tt=Tt,
                                  sbase=b["base"] + t0,
                                  first=(k0 == 0), last=(k0 + Kp == K),
                                  tid=tid, bidx=bi, t0=t0, k0=k0))
                col += Kp * Tt
                k0 += Kp
            tid += 1
    assert col == p.LT
    p.units = units

    # chunk packing: greedy, boundaries between units
    chunks = []
    cur_u, cur0 = [], 0
    for ui, u in enumerate(units):
        ucols = u["Kp"] * u["Tt"]
        cap = HEAD_CAP if cur0 < HEAD_N * HEAD_CAP else CHUNK
        if u["col"] + ucols - cur0 > cap:
            chunks.append((cur0, u["col"] - cur0, cur_u))
            cur_u, cur0 = [], u["col"]
        cur_u.append(ui)
    if cur_u:
        chunks.append((cur0, p.LT - cur0, cur_u))
    p.chunks = chunks

    # per-core slot -> segment map
    p.slot_seg = np.full((NCORES, p.S), -1, np.int64)
    p.slot_cnt = np.zeros((NCORES, p.S), np.int64)
    for c in range(NCORES):
        sc_ = segs_c[c]
        Ksc = Kof[sc_]
        for b in p.buckets:
            segs = sc_[Ksc == b["K"]]
            p.slot_seg[c, b["base"]:b["base"] + len(segs)] = segs
            p.slot_cnt[c, b["base"]:b["base"] + len(segs)] = counts[segs]
    p.n_inv = (p.slot_seg < 0).sum(axis=1)

    p.counts = counts
    p.order_e = np.argsort(index, kind="stable")
    p.starts = np.zeros(N + 1, np.int64)
    np.cumsum(counts, out=p.starts[1:])

    p.nSC = -(-p.S // SC)
    return p


def make_core_arrays(p, c, x_bf):
    """xt [128, LT] bf16 slab-major stream (layout mirrors p.units)."""
    E = p.E
    eT = np.full(p.LT, E, np.int64)
    Ms = []
    for b in p.buckets:
        K, SK, base = b["K"], b["SK"], b["base"]
        cnts = p.slot_cnt[c, base:base + SK]
        segs = p.slot_seg[c, base:base + SK]
        M = np.full((SK, K), E, np.int64)
        tot = int(cnts.sum())
        if tot:
            rr = np.repeat(np.arange(SK), cnts)
            cum0 = np.concatenate(([0], np.cumsum(cnts)[:-1]))
            cc = np.arange(tot) - np.repeat(cum0, cnts)
            src = p.order_e[np.repeat(p.starts[np.maximum(segs, 0)], cnts) + cc]
            M[rr, cc] = src
        Ms.append(M)
    for u in p.units:
        M = Ms[u["bidx"]]
        t0, k0, Kp, Tt = u["t0"], u["k0"], u["Kp"], u["Tt"]
        eT[u["col"]:u["col"] + Kp * Tt] = \
            M[t0:t0 + Tt, k0:k0 + Kp].T.ravel()
    xt = np.ascontiguousarray(x_bf[eT].T)
    return xt


# ----------------------------------------------------------------------------
# Device kernel
# ----------------------------------------------------------------------------

def build_kernel(p):
    nc = bacc.Bacc("TRN2", target_bir_lowering=False, debug=False,
                   num_devices=NCORES)
    S, LT = p.S, p.LT

    xt_d = nc.dram_tensor("xt", [128, LT], dt.bfloat16, kind="ExternalInput")
    rcb_d = nc.dram_tensor("rcb", [128, S], dt.bfloat16, kind="ExternalInput")
    emb_d = nc.dram_tensor("embT", [128, S], dt.bfloat16, kind="ExternalInput")
    w5_d = nc.dram_tensor("w5", [128, 5 * 128], dt.bfloat16, kind="ExternalInput")
    gamma_d = nc.dram_tensor("gamma", [128, 1], dt.float32, kind="ExternalInput")
    beta_d = nc.dram_tensor("beta", [128, 1], dt.float32, kind="ExternalInput")
    ident_d = nc.dram_tensor("ident128", [128, 128], dt.bfloat16, kind="ExternalInput")
    hinv_d = nc.dram_tensor("hinv", [128, 1], dt.float32, kind="ExternalInput")
    ninv_d = nc.dram_tensor("ninv", [128, 1], dt.float32, kind="ExternalInput")
    hout_d = nc.dram_tensor("hout", [128, S], dt.bfloat16, kind="ExternalOutput")

    units, chunks = p.units, p.chunks
    A = mybir.AluOpType
    AF = mybir.ActivationFunctionType

    with tile.TileContext(nc) as tc:
        import contextlib
        with contextlib.ExitStack() as ctx:
            cpool = ctx.enter_context(tc.tile_pool(name="const", bufs=1))
            stpool = ctx.enter_context(tc.tile_pool(name="stats", bufs=1))
            tpool = ctx.enter_context(tc.tile_pool(name="tchunk", bufs=2))
            qpool = ctx.enter_context(tc.tile_pool(name="sqchunk", bufs=2))
            fpool = ctx.enter_context(tc.tile_pool(name="ftmp", bufs=1))
            spool = ctx.enter_context(tc.tile_pool(name="stage", bufs=2))
            pss = ctx.enter_context(tc.tile_pool(name="pss", bufs=3, space="PSUM"))
            psq = ctx.enter_context(tc.tile_pool(name="psq", bufs=3, space="PSUM"))
            psh = ctx.enter_context(tc.tile_pool(name="psh", bufs=1, space="PSUM"))
            dram = ctx.enter_context(tc.tile_pool(name="dram", bufs=1, space="DRAM"))

            # ---- first stream chunks (issued before constants so the
            # folds can start as early as possible) ----
            pre_tch = []
            for (c0, ncols, _u) in chunks[:2]:
                t = tpool.tile([128, CHUNK], dt.bfloat16, tag="tch")
                nc.sync.dma_start(t[:, 0:ncols], xt_d.ap()[:, c0:c0 + ncols])
                pre_tch.append(t)

            # ---- constants ----
            ident = cpool.tile([128, 128], dt.bfloat16, tag="ident")
            nc.sync.dma_start(ident[:], ident_d.ap())
            w5 = cpool.tile([128, 5 * 128], dt.bfloat16, tag="w5")
            nc.sync.dma_start(w5[:], w5_d.ap())
            gamma = cpool.tile([128, 1], dt.float32, tag="gamma")
            nc.sync.dma_start(gamma[:], gamma_d.ap())
            beta = cpool.tile([128, 1], dt.float32, tag="beta")
            nc.sync.dma_start(beta[:], beta_d.ap())
            hinv = cpool.tile([128, 1], dt.float32, tag="hinv")
            nc.sync.dma_start(hinv[:], hinv_d.ap())
            ninv = cpool.tile([128, 1], dt.float32, tag="ninv")
            nc.sync.dma_start(ninv[:], ninv_d.ap())

            # ---- persistent stats / tables ----
            mnT = stpool.tile([128, S], dt.bfloat16, tag="mnT")
            mxT = stpool.tile([128, S], dt.bfloat16, tag="mxT")
            meanT = stpool.tile([128, S], dt.bfloat16, tag="meanT")
            sqT = stpool.tile([128, S], dt.bfloat16, tag="sqT")
            hm = stpool.tile([128, S], dt.bfloat16, tag="hm")
            rcb = stpool.tile([128, S], dt.bfloat16, tag="rcb")
            nc.gpsimd.dma_start(rcb[:], rcb_d.ap())
            embT = stpool.tile([128, S], dt.bfloat16, tag="embT")
            nc.gpsimd.dma_start(embT[:], emb_d.ap())
            smp = stpool.tile([128, p.nSC], dt.float32, tag="smp")
            sqp = stpool.tile([128, p.nSC], dt.float32, tag="sqp")

            # ---- fold helper ----
            fv = fpool.tile([128, CHUNK // 2], dt.bfloat16, tag="fv")
            fg = fpool.tile([128, CHUNK // 2], dt.bfloat16, tag="fg")

            def emit_fold(eng, tmp, tch, off, Kp, Tt, dest, sbase, first, op):
                w = Kp
                cur = tch
                cbase = off
                while True:
                    half = (w + 1) // 2
                    nf = (w - half) * Tt
                    i0 = cur[:, cbase:cbase + nf]
                    i1 = cur[:, cbase + half * Tt:cbase + w * Tt]
                    if half == 1:
                        if first:
                            o = dest[:, sbase:sbase + Tt]
                        else:
                            o = tmp[:, 0:Tt]
                        eng.tensor_tensor(out=o, in0=i0, in1=i1, op=op)
                        break
                    eng.tensor_tensor(out=tmp[:, 0:nf], in0=i0, in1=i1, op=op)
                    cur, cbase, w = tmp, 0, half
                if not first:
                    eng.tensor_tensor(out=dest[:, sbase:sbase + Tt],
                                      in0=dest[:, sbase:sbase + Tt],
                                      in1=tmp[:, 0:Tt], op=op)

            # ---- MLP chunk ----
            def emit_mlp(ci):
                o0 = ci * SC
                cw = min(SC, S - o0)
                sl = slice(o0, o0 + cw)
                # scale raw sums -> mean, msq
                nc.vector.tensor_tensor(out=meanT[:, sl], in0=meanT[:, sl],
                                        in1=rcb[:, sl], op=A.mult)
                nc.vector.tensor_tensor(out=sqT[:, sl], in0=sqT[:, sl],
                                        in1=rcb[:, sl], op=A.mult)
                # std
                vt = spool.tile([128, SC], dt.bfloat16, tag="vt")
                nc.vector.tensor_tensor(out=vt[:, 0:cw], in0=meanT[:, sl],
                                        in1=meanT[:, sl], op=A.mult)
                nc.vector.tensor_tensor(out=vt[:, 0:cw], in0=sqT[:, sl],
                                        in1=vt[:, 0:cw], op=A.subtract)
                nc.vector.tensor_scalar(out=vt[:, 0:cw], in0=vt[:, 0:cw],
                                        scalar1=0.0, scalar2=EPS_STD,
                                        op0=A.max, op1=A.add)
                nc.scalar.activation(out=sqT[:, sl], in_=vt[:, 0:cw],
                                     func=AF.Sqrt)
                # h = sum_k W_k^T @ stat_k
                ph = psh.tile([128, SC], dt.float32, tag="ph")
                stats = (meanT, mnT, mxT, sqT, embT)
                for h0 in range(0, cw, 512):
                    hw = min(512, cw - h0)
                    for k in range(5):
                        nc.tensor.matmul(out=ph[:, h0:h0 + hw],
                                         lhsT=w5[:, k * 128:(k + 1) * 128],
                                         rhs=stats[k][:, o0 + h0:o0 + h0 + hw],
                                         start=(k == 0), stop=(k == 4))
                # hm + BN partials
                nc.scalar.activation(out=hm[:, sl], in_=ph[:, 0:cw],
                                     func=AF.Copy,
                                     accum_out=smp[:, ci:ci + 1])
                hsq = spool.tile([128, SC], dt.bfloat16, tag="hsq")
                nc.scalar.activation(out=hsq[:, 0:cw], in_=hm[:, sl],
                                     func=AF.Square,
                                     accum_out=sqp[:, ci:ci + 1])

            # ---- main loop ----
            wsum, wsq = {}, {}
            mlp_done = 0
            fin_slot = [0]

            def close_tile(u):
                b_sbase, Tt = u["sbase"], u["Tt"]
                ps = wsum.pop(u["tid"])
                nc.scalar.copy(out=meanT[:, b_sbase:b_sbase + Tt],
                               in_=ps[:, 0:Tt])
                ps2 = wsq.pop(u["tid"])
                nc.scalar.copy(out=sqT[:, b_sbase:b_sbase + Tt],
                               in_=ps2[:, 0:Tt])
                fin_slot[0] = b_sbase + Tt

            for ch_i, (c0, ncols, uids) in enumerate(chunks):
                if ch_i < len(pre_tch):
                    tch = pre_tch[ch_i]
                else:
                    tch = tpool.tile([128, CHUNK], dt.bfloat16, tag="tch")
                    nc.sync.dma_start(tch[:, 0:ncols],
                                      xt_d.ap()[:, c0:c0 + ncols])
                sq = qpool.tile([128, CHUNK], dt.bfloat16, tag="sq")
                half = (ncols // 2) & ~1
                nc.scalar.activation(out=sq[:, 0:half], in_=tch[:, 0:half],
                                     func=AF.Square)
                nc.scalar.activation(out=sq[:, half:ncols],
                                     in_=tch[:, half:ncols], func=AF.Square)
                for ui in uids:
                    u = units[ui]
                    off = u["col"] - c0
                    Kp, Tt = u["Kp"], u["Tt"]
                    if u["first"]:
                        wsum[u["tid"]] = pss.tile([128, T_S], dt.float32,
                                                  tag="pssum", name="pssum")
                        wsq[u["tid"]] = psq.tile([128, T_S], dt.float32,
                                                 tag="pssq", name="pssq")
                    ps = wsum[u["tid"]]
                    ps2 = wsq[u["tid"]]
                    for j in range(Kp):
                        nc.tensor.matmul(
                            out=ps[:, 0:Tt], lhsT=ident[:],
                            rhs=tch[:, off + j * Tt:off + (j + 1) * Tt],
                            start=(u["first"] and j == 0),
                            stop=(u["last"] and j == Kp - 1))
                    emit_fold(nc.vector, fv, tch, off, Kp, Tt, mnT,
                              u["sbase"], u["first"], A.min)
                    emit_fold(nc.vector, fg, tch, off, Kp, Tt, mxT,
                              u["sbase"], u["first"], A.max)
                    for j in range(Kp):
                        nc.tensor.matmul(
                            out=ps2[:, 0:Tt], lhsT=ident[:],
                            rhs=sq[:, off + j * Tt:off + (j + 1) * Tt],
                            start=(u["first"] and j == 0),
                            stop=(u["last"] and j == Kp - 1))
                    if u["last"]:
                        close_tile(u)
                # interleave MLP chunks whose stats are final
                while mlp_done < p.nSC and (mlp_done + 1) * SC <= fin_slot[0]:
                    emit_mlp(mlp_done)
                    mlp_done += 1
            while mlp_done < p.nSC:
                emit_mlp(mlp_done)
                mlp_done += 1

            # ---- BN stats + correction + AllReduce ----
            bn = spool.tile([128, 2], dt.float32, tag="bn")
            nc.vector.tensor_reduce(out=bn[:, 0:1], in_=smp[:],
                                    axis=mybir.AxisListType.X, op=A.add)
            nc.vector.tensor_reduce(out=bn[:, 1:2], in_=sqp[:],
                                    axis=mybir.AxisListType.X, op=A.add)
            hinv2 = spool.tile([128, 1], dt.float32, tag="hinv2")
            nc.scalar.activation(out=hinv2[:], in_=hinv[:], func=AF.Square)
            corr = spool.tile([128, 2], dt.float32, tag="corr")
            nc.vector.tensor_scalar(out=corr[:, 0:1], in0=hinv[:],
                                    scalar1=ninv[:], scalar2=None, op0=A.mult)
            nc.vector.tensor_scalar(out=corr[:, 1:2], in0=hinv2[:],
                                    scalar1=ninv[:], scalar2=None, op0=A.mult)
            nc.vector.tensor_tensor(out=bn[:], in0=bn[:], in1=corr[:],
                                    op=A.subtract)

            bno = spool.tile([128, 2], dt.float32, tag="bno")
            if USE_ALLGATHER:
                gi = dram.tile([128, 2], dt.float32)
                go = dram.tile([NCORES * 128, 2], dt.float32)
                nc.gpsimd.dma_start(gi[:], bn[:])
                nc.gpsimd.collective_compute(
                    "AllGather", mybir.AluOpType.bypass,
                    replica_groups=[list(range(NCORES))],
                    ins=[gi.opt()], outs=[go.opt()])
                gath = spool.tile([128, 2 * NCORES], dt.float32, tag="gath")
                nc.gpsimd.dma_start(
                    gath[:].rearrange("p (r c) -> p r c", c=2),
                    go[:].rearrange("(r p) c -> p r c", p=128))
                nc.vector.tensor_reduce(
                    out=bno[:],
                    in_=gath[:].rearrange("p (r c) -> p c r", c=2),
                    axis=mybir.AxisListType.X, op=A.add)
            else:
                bounce_i = dram.tile([128, 2], dt.float32)
                bounce_o = dram.tile([128, 2], dt.float32)
                nc.gpsimd.dma_start(bounce_i[:], bn[:])
                nc.gpsimd.collective_compute(
                    "AllReduce", mybir.AluOpType.add,
                    replica_groups=[list(range(NCORES))],
                    ins=[bounce_i.opt()], outs=[bounce_o.opt()])
                nc.gpsimd.dma_start(bno[:], bounce_o[:])

            inv_n = 1.0 / float(p.N)
            mu = spool.tile([128, 1], dt.float32, tag="mu")
            nc.vector.tensor_scalar(out=mu[:], in0=bno[:, 0:1],
                                    scalar1=inv_n, scalar2=None, op0=A.mult)
            ex2 = spool.tile([128, 1], dt.float32, tag="ex2")
            nc.vector.tensor_scalar(out=ex2[:], in0=bno[:, 1:2],
                                    scalar1=inv_n, scalar2=None, op0=A.mult)
            var = spool.tile([128, 1], dt.float32, tag="var")
            nc.vector.tensor_tensor(out=var[:], in0=mu[:], in1=mu[:],
                                    op=A.mult)
            nc.vector.tensor_tensor(out=var[:], in0=ex2[:], in1=var[:],
                                    op=A.subtract)
            nc.vector.tensor_scalar(out=var[:], in0=var[:], scalar1=EPS_BN,
                                    scalar2=None, op0=A.add)
            sdv = spool.tile([128, 1], dt.float32, tag="sdv")
            nc.scalar.activation(out=sdv[:], in_=var[:], func=AF.Sqrt)
            istd = spool.tile([128, 1], dt.float32, tag="istd")
            nc.vector.reciprocal(out=istd[:], in_=sdv[:])
            scl = spool.tile([128, 1], dt.float32, tag="scl")
            nc.vector.tensor_tensor(out=scl[:], in0=gamma[:], in1=istd[:],
                                    op=A.mult)
            shf = spool.tile([128, 1], dt.float32, tag="shf")
            nc.vector.tensor_tensor(out=shf[:], in0=mu[:], in1=scl[:],
                                    op=A.mult)
            nc.vector.tensor_tensor(out=shf[:], in0=beta[:], in1=shf[:],
                                    op=A.subtract)

            # ---- normalize + relu + out (8 blocks, ACT/DVE split) ----
            OB = -(-S // 8)
            OB += OB & 1
            for ci in range(8):
                o0 = ci * OB
                cw = min(OB, S - o0)
                if cw <= 0:
                    break
                hs = spool.tile([128, OB], dt.bfloat16, tag="hs")
                if ci % 2 == 0:
                    nc.scalar.activation(out=hs[:, 0:cw],
                                         in_=hm[:, o0:o0 + cw],
                                         func=AF.Relu, scale=scl[:],
                                         bias=shf[:])
                else:
                    nc.vector.tensor_scalar(out=hs[:, 0:cw],
                                            in0=hm[:, o0:o0 + cw],
                                            scalar1=scl[:], scalar2=shf[:],
                                            op0=A.mult, op1=A.add)
                    nc.vector.tensor_scalar(out=hs[:, 0:cw], in0=hs[:, 0:cw],
                                            scalar1=0.0, scalar2=None,
                                            op0=A.max)
                nc.sync.dma_start(hout_d.ap()[:, o0:o0 + cw], hs[:, 0:cw])

    nc.compile()
    return nc


# ----------------------------------------------------------------------------
# Top-level
# ----------------------------------------------------------------------------

def prepare(inputs, index, deg_emb, W, gamma, beta, dim_size):
    N = int(dim_size)
    E = index.shape[0]
    index = np.asarray(index)
    p = make_plan(index, N)

    x_bf = np.empty((E + 1, 128), BF16)
    x_bf[:E] = np.asarray(inputs).astype(BF16)
    x_bf[E] = 0

    W64 = np.asarray(W, dtype=np.float64)
    demb64 = np.asarray(deg_emb, dtype=np.float64)
    # h of an invalid slot: stats 0, std = sqrt(eps), emb = demb[0]
    hinv = (np.sqrt(EPS_STD) * W64[3 * 128:4 * 128].sum(axis=0)
            + demb64[0] @ W64[4 * 128:5 * 128]).astype(F32)

    demb_bf = np.asarray(deg_emb).astype(BF16)
    in_maps = []
    for c in range(NCORES):
        xt = make_core_arrays(p, c, x_bf)
        cnt = p.slot_cnt[c]
        rc = (1.0 / np.maximum(cnt, 1)).astype(BF16)
        deg = np.minimum(cnt, 99).astype(np.int64)
        embT = np.ascontiguousarray(demb_bf[deg].T)
        m = {
            "xt": xt,
            "rcb": np.ascontiguousarray(np.broadcast_to(rc, (128, p.S))),
            "embT": embT,
            "w5": np.ascontiguousarray(
                np.asarray(W).astype(BF16).reshape(5, 128, 128)
                .transpose(1, 0, 2).reshape(128, 5 * 128)),
            "gamma": np.asarray(gamma).astype(F32).reshape(128, 1),
            "beta": np.asarray(beta).astype(F32).reshape(128, 1),
            "ident128": np.eye(128, dtype=BF16),
            "hinv": hinv.reshape(128, 1),
            "ninv": np.full((128, 1), float(p.n_inv[c]), F32),
        }
        in_maps.append(m)

    nc = build_kernel(p)
    prepare.last_plan = p

    def assemble(results):
        out = np.zeros((N, 128), F32)
        for c in range(NCORES):
            hT = results[c]["hout"].astype(F32)  # [128, S]
            segs = p.slot_seg[c]
            mask = segs >= 0
            out[segs[mask]] = hT.T[mask]
        return out

    return nc, in_maps, assemble


def kernel(inputs, index, deg_emb, W, gamma, beta, dim_size):
    nc, in_maps, assemble = prepare(inputs, index, deg_emb, W, gamma, beta,
                                    dim_size)
    res = bass_utils.run_bass_kernel_spmd(
        nc, in_maps, core_ids=list(range(NCORES)))
    return assemble(res.results)
